# revision 23
# baseline (speedup 1.0000x reference)
"""Trainium2 Bass kernel for nn_ControlNet: out = x @ W^T + bias.

Shapes: x [64, 128, 128] f32, weight [16384, 16384] f32, bias [16384] f32.

Fast path (used when the weight matches the ControlNet structure,
verified on the host): the weight is a fixed block-diagonal 0/1 scatter
with 128 identical blocks -- each output row has at most one source:

  out[b, n, c] = x[b, n, 183 - c] + bias[n, c]   for c in [93, 120)
  out[b, n, c] = bias[n, c]                      otherwise

so the op is pure data movement plus one 27-column add; no matmul.
Sharding: data-parallel over batch (8 batches/core). Each core receives
only the 27-col band of x it needs plus bias (fp16, one 86KB DMA),
builds out[n, b, m] = broadcast(bias) on DVE with the band added on top
(fp16, stride-0 broadcast APs), and stores the 256KB fp16 result in two
batch-hunks on separate DMA queues (a tiny dummy store pre-warms the
second queue; a cold queue costs ~1us to first data). The host casts
back to f32 and unshards. Measured ~14.9us vs 268.5us for the dense
fp16 matmul baseline (fixed NEFF scaffold is ~10.5us of that); max rel
err ~5e-4 from fp16 rounding (gate: 2e-2).

Fallback (any other weight): dense tensor-parallel row-shard of W^T
across 8 cores streaming through the PE array -- fp16 with an exact
two-term hi/lo split of x when the weight is fp16-representable, else
float32r with the same split. See _build_nc_fp16/_build_nc_f32r.
"""

import numpy as np

import concourse.bacc as bacc
import concourse.bass as bass
import concourse.mybir as mybir
import concourse.tile as tile
from concourse.bass_utils import run_bass_kernel_spmd

BATCH = 64
NM = 128 * 128          # 16384 flattened features
N_CORES = 8
O_SHARD = NM // N_CORES  # 2048 output features per core
K_CHUNK = 128            # contraction handled 128 rows (partitions) at a time
N_KCHUNKS = NM // K_CHUNK  # 128
MM_FREE = 512            # psum bank limit: 512 fp32 outputs per matmul
N_OCHUNKS = O_SHARD // MM_FREE  # 4
LO_SHIFT = 11            # x_lo scale: 2^11 (fp16 mantissa width)

F32 = mybir.dt.float32
F32R = mybir.dt.float32r
F16 = mybir.dt.float16

_compiled = {}

# --- fast path: the ControlNet weight is a fixed block-diagonal 0/1 scatter:
#   out[b, n, c] = x[b, n, 183 - c] + bias[n, c]   for c in [93, 120)
#   out[b, n, c] = bias[n, c]                      otherwise
# (128 identical blocks; 27 ones per block; 3456 nonzeros total).
# No matmul needed: per core (batch-shard of 8) the device reads the 27-col
# band of x + bias (one 86 KB fp16 DMA), builds out = broadcast(bias) with
# the band added on DVE, and stores 256 KB fp16. The single-shot NEFF time
# is dominated by the ~10.5us fixed scaffold (start barrier, per-engine
# library loads, exit barriers); the body is ~4.5us.
BAND_LO = 93
BAND_HI = 120
BAND_W = BAND_HI - BAND_LO  # 27
B_CORE = BATCH // N_CORES   # 8 batches per core


def _expected_pattern():
    n = np.repeat(np.arange(128), BAND_W)
    j = np.tile(np.arange(37, 64), 128)
    return 128 * n + j + 56, 128 * n + 127 - j


def _is_controlnet_weight(weight):
    if weight.shape != (NM, NM):
        return False
    if np.count_nonzero(weight) != 3456:
        return False
    oi, ii = _expected_pattern()
    return bool(np.all(weight[oi, ii] == 1.0))


def _build_nc_v4(nh=2, out_dt=F16, in_dt=F16, split_input=False,
                 store_mode="alt", num_devices=N_CORES):
    """All-fp16 data path: fp16 inputs (band 216 cols + bias 128 cols in
    one DRAM tensor, one DMA on the sync ring), fp16 DVE compute (2x
    rate), fp16 output (half the store bytes). nh batch-hunks, stores
    alternating sync/scalar rings. store_mode "warm" issues a tiny dummy
    store on scalar first so its DMA queue is warm before the real store
    (a cold queue costs ~1us to first data; measured best + most stable:
    nh=2 + warm). split_input / sync2 were slower in HW sweeps.
    """
    hb = B_CORE // nh
    nc = bacc.Bacc("TRN2", target_bir_lowering=False, debug=False,
                   num_devices=num_devices)
    BCOLS = B_CORE * BAND_W  # 216
    in_d = nc.dram_tensor("inp", [128, BCOLS + 128], in_dt,
                          kind="ExternalInput")
    out_d = nc.dram_tensor("out", [128, B_CORE, 128], out_dt,
                           kind="ExternalOutput")
    if store_mode == "warm":
        scratch_d = nc.dram_tensor("scratch", [1, 16], in_dt,
                                   kind="Internal")

    with tile.TileContext(nc) as tc:
        with tc.tile_pool(name="pool", bufs=1) as pool:
            in_sb = pool.tile([128, BCOLS + 128], in_dt)
            if store_mode == "warm":
                # tiny write to warm the scalar DMA queue before the
                # real store needs it (cold queue: ~1us to first data)
                warm_sb = pool.tile([1, 16], in_dt)
                nc.vector.memset(warm_sb[:], 0.0)
                nc.scalar.dma_start(scratch_d.ap(), warm_sb[:])
            if split_input:
                nc.sync.dma_start(in_sb[:, BCOLS:BCOLS + 128],
                                  in_d.ap()[:, BCOLS:BCOLS + 128])
                nc.scalar.dma_start(in_sb[:, 0:BCOLS],
                                    in_d.ap()[:, 0:BCOLS])
            else:
                nc.sync.dma_start(in_sb[:], in_d.ap())
            band = in_sb[:, 0:BCOLS].rearrange("p (b k) -> p b k", b=B_CORE)
            bias = in_sb[:, BCOLS:BCOLS + 128]
            if store_mode == "sync2":
                rings = [nc.sync, nc.sync]
            else:
                rings = [nc.sync, nc.scalar]

            out_sb = pool.tile([128, B_CORE, 128], out_dt)
            for h in range(nh):
                bsl = slice(h * hb, (h + 1) * hb)
                nc.vector.tensor_copy(
                    out_sb[:, bsl, :],
                    bias.unsqueeze(1).broadcast_to([128, hb, 128]))
                nc.vector.tensor_add(
                    out_sb[:, bsl, BAND_LO:BAND_HI],
                    band[:, bsl, :],
                    bias[:, BAND_LO:BAND_HI].unsqueeze(1)
                    .broadcast_to([128, hb, BAND_W]))
                rings[h % 2].dma_start(out_d.ap()[:, bsl, :],
                                       out_sb[:, bsl, :])

    nc.compile()
    return nc


def _v4_in_maps(x, bias, in_np=np.float16):
    band = x[:, :, 90:63:-1]  # [64, 128, 27]
    bias2d = bias.reshape(128, 128)
    maps = []
    for c in range(N_CORES):
        bc = band[c * B_CORE:(c + 1) * B_CORE].transpose(1, 0, 2)  # [128,8,27]
        inp = np.empty((128, B_CORE * BAND_W + 128), dtype=in_np)
        inp[:, 0:B_CORE * BAND_W] = bc.reshape(128, -1)
        inp[:, B_CORE * BAND_W:] = bias2d
        maps.append({"inp": inp})
    return maps


def _v4_unshard(results):
    out = np.concatenate(
        [r["out"].astype(np.float32).transpose(1, 0, 2) for r in results],
        axis=0)
    return np.ascontiguousarray(out)


def _common_io(nc, mm_dt, g, bias_dt):
    n_groups = N_KCHUNKS // g
    xh_d = nc.dram_tensor("xh", [K_CHUNK, N_KCHUNKS * BATCH], mm_dt,
                          kind="ExternalInput")
    xl_d = nc.dram_tensor("xl", [K_CHUNK, N_KCHUNKS * BATCH], mm_dt,
                          kind="ExternalInput")
    wt_d = nc.dram_tensor("wt", [NM, O_SHARD], mm_dt, kind="ExternalInput")
    bias_d = nc.dram_tensor("bias", [2, O_SHARD], bias_dt,
                            kind="ExternalInput")
    out_d = nc.dram_tensor("out", [BATCH, O_SHARD], F32, kind="ExternalOutput")
    # W^T shard grouped for DMA: k = (g_idx*g + j)*128 + p  ->  [g_idx, p, j, o]
    wt_view = wt_d.ap().rearrange("(g j p) o -> g p j o", g=n_groups, j=g,
                                  p=K_CHUNK)
    return xh_d, xl_d, wt_view, bias_d, out_d


def _build_nc_fp16(g=8, wbufs=3, repeat=1):
    """fp16 W + exact fp16 hi/lo split of x, two PSUM chains.

    Every PE instruction is fp16 (the fp32/fp16 mix crashed the exec
    unit): bias is split like x, bias_hi into the hi chain and
    bias_lo * 2^11 into the lo chain, each as the chain-starting
    contract-dim-1 matmul.

    repeat > 1 wraps the streaming body in a device-side For_i loop —
    used only for benchmarking (per-call dispatch overhead through the
    axon tunnel is ~88 ms, so single executions can't be timed).
    """
    n_groups = N_KCHUNKS // g
    nc = bacc.Bacc("TRN2", target_bir_lowering=False, debug=False,
                   num_devices=N_CORES)
    xh_d, xl_d, wt_view, bias_d, out_d = _common_io(nc, F16, g, F16)

    with tile.TileContext(nc) as tc:
        with (
            tc.tile_pool(name="const", bufs=1) as const_pool,
            tc.tile_pool(name="wpool", bufs=wbufs) as wpool,
            tc.tile_pool(name="psum", bufs=1, space=bass.MemorySpace.PSUM) as psum_pool,
            tc.tile_pool(name="opool", bufs=1) as opool,
        ):
            xh_sb = const_pool.tile([K_CHUNK, N_KCHUNKS * BATCH], F16)
            nc.sync.dma_start(xh_sb[:], xh_d.ap())
            xl_sb = const_pool.tile([K_CHUNK, N_KCHUNKS * BATCH], F16)
            nc.sync.dma_start(xl_sb[:], xl_d.ap())
            bias_hi_sb = const_pool.tile([1, O_SHARD], F16)
            nc.sync.dma_start(bias_hi_sb[:], bias_d.ap()[0:1])
            bias_lo_sb = const_pool.tile([1, O_SHARD], F16)
            nc.sync.dma_start(bias_lo_sb[:], bias_d.ap()[1:2])
            ones_sb = const_pool.tile([1, BATCH], F16)
            nc.vector.memset(ones_sb[:], 1.0)

            def body():
                psum_hi = psum_pool.tile([BATCH, O_SHARD], F32, tag="ph")
                psum_lo = psum_pool.tile([BATCH, O_SHARD], F32, tag="pl")
                # bias rows into each chain: [1,64].T @ [1,512] outer product
                for oc in range(N_OCHUNKS):
                    sl = slice(oc * MM_FREE, (oc + 1) * MM_FREE)
                    nc.tensor.matmul(psum_hi[:, sl], ones_sb[:, :],
                                     bias_hi_sb[0:1, sl], start=True, stop=False)
                    nc.tensor.matmul(psum_lo[:, sl], ones_sb[:, :],
                                     bias_lo_sb[0:1, sl], start=True, stop=False)

                for g_idx in range(n_groups):
                    w_sb = wpool.tile([K_CHUNK, g, O_SHARD], F16, tag="w")
                    nc.sync.dma_start(w_sb[:], wt_view[g_idx])
                    for j in range(g):
                        c = g_idx * g + j
                        lhs_hi = xh_sb[:, c * BATCH:(c + 1) * BATCH]
                        lhs_lo = xl_sb[:, c * BATCH:(c + 1) * BATCH]
                        last = c == N_KCHUNKS - 1
                        for oc in range(N_OCHUNKS):
                            rhs = w_sb[:, j, oc * MM_FREE:(oc + 1) * MM_FREE]
                            sl = slice(oc * MM_FREE, (oc + 1) * MM_FREE)
                            nc.tensor.matmul(psum_hi[:, sl], lhs_hi, rhs,
                                             start=False, stop=last)
                            nc.tensor.matmul(psum_lo[:, sl], lhs_lo, rhs,
                                             start=False, stop=last)

                out_sb = opool.tile([BATCH, O_SHARD], F32, tag="o")
                # out = (lo * 2^-11) + hi (DVE reads <=1 PSUM input per op)
                nc.vector.tensor_scalar_mul(out_sb[:], psum_lo[:],
                                            2.0 ** -LO_SHIFT)
                nc.vector.tensor_add(out_sb[:], out_sb[:], psum_hi[:])
                nc.sync.dma_start(out_d.ap(), out_sb[:])

            if repeat == 1:
                body()
            else:
                with tc.For_i(0, repeat, 1):
                    body()

    nc.compile()
    return nc


def _build_nc_fp16ct(g=8, wbufs=3, repeat=1, const_engine=None, dual_ring=False):
    """Column-tiled fp16 variant: hi chain on PE columns 0-63
    (tile_position (0,0), PSUM partitions 0-63), lo chain on columns
    64-127 (tile_position (0,64), PSUM partitions 64-127). The two
    matmuls of each k-chunk run concurrently on disjoint column groups,
    halving effective PE time. The tail merges across partitions with an
    SBUF->SBUF accumulate DMA (SWDGE)."""
    n_groups = N_KCHUNKS // g
    nc = bacc.Bacc("TRN2", target_bir_lowering=False, debug=False,
                   num_devices=N_CORES)
    xh_d, xl_d, wt_view, bias_d, out_d = _common_io(nc, F16, g, F16)

    with tile.TileContext(nc) as tc:
        with (
            tc.tile_pool(name="const", bufs=1) as const_pool,
            tc.tile_pool(name="wpool", bufs=wbufs) as wpool,
            tc.tile_pool(name="psum", bufs=1, space=bass.MemorySpace.PSUM) as psum_pool,
            tc.tile_pool(name="opool", bufs=1) as opool,
        ):
            ce = nc.scalar if const_engine == "scalar" else nc.sync
            xh_sb = const_pool.tile([K_CHUNK, N_KCHUNKS * BATCH], F16)
            ce.dma_start(xh_sb[:], xh_d.ap())
            xl_sb = const_pool.tile([K_CHUNK, N_KCHUNKS * BATCH], F16)
            ce.dma_start(xl_sb[:], xl_d.ap())
            bias_hi_sb = const_pool.tile([1, O_SHARD], F16)
            ce.dma_start(bias_hi_sb[:], bias_d.ap()[0:1])
            bias_lo_sb = const_pool.tile([1, O_SHARD], F16)
            ce.dma_start(bias_lo_sb[:], bias_d.ap()[1:2])
            ones_sb = const_pool.tile([1, BATCH], F16)
            nc.vector.memset(ones_sb[:], 1.0)

            def body():
                # separate banks per chain: hi banks 0-3 (partitions 0-63),
                # lo banks 4-7 (partitions 64-127, via col-group 2-3)
                psum_hi = psum_pool.tile([BATCH, O_SHARD], F32, tag="ph")
                psum_lo = psum_pool.tile([2 * BATCH, O_SHARD], F32, tag="pl")
                for oc in range(N_OCHUNKS):
                    sl = slice(oc * MM_FREE, (oc + 1) * MM_FREE)
                    nc.tensor.matmul(psum_hi[:, sl], ones_sb[:, :],
                                     bias_hi_sb[0:1, sl], start=True,
                                     stop=False, tile_position=(0, 0))
                    nc.tensor.matmul(psum_lo[BATCH:2 * BATCH, sl],
                                     ones_sb[:, :],
                                     bias_lo_sb[0:1, sl], start=True,
                                     stop=False, tile_position=(0, 64))

                for g_idx in range(n_groups):
                    w_sb = wpool.tile([K_CHUNK, g, O_SHARD], F16, tag="w")
                    weng = (nc.scalar if (dual_ring and g_idx % 2) else nc.sync)
                    weng.dma_start(w_sb[:], wt_view[g_idx])
                    for j in range(g):
                        c = g_idx * g + j
                        lhs_hi = xh_sb[:, c * BATCH:(c + 1) * BATCH]
                        lhs_lo = xl_sb[:, c * BATCH:(c + 1) * BATCH]
                        last = c == N_KCHUNKS - 1
                        for oc in range(N_OCHUNKS):
                            rhs = w_sb[:, j, oc * MM_FREE:(oc + 1) * MM_FREE]
                            sl = slice(oc * MM_FREE, (oc + 1) * MM_FREE)
                            nc.tensor.matmul(psum_hi[:, sl], lhs_hi, rhs,
                                             start=False, stop=last,
                                             tile_position=(0, 0))
                            nc.tensor.matmul(psum_lo[BATCH:2 * BATCH, sl],
                                             lhs_lo, rhs,
                                             start=False, stop=last,
                                             tile_position=(0, 64))

                out_sb = opool.tile([2 * BATCH, O_SHARD], F32, tag="o")
                # rows 64-127: lo * 2^-11 ; rows 0-63: hi
                nc.vector.tensor_scalar_mul(out_sb[BATCH:2 * BATCH, :],
                                            psum_lo[BATCH:2 * BATCH, :],
                                            2.0 ** -LO_SHIFT)
                nc.vector.tensor_copy(out_sb[0:BATCH, :], psum_hi[:, :])
                # cross-partition merge: out[0:64] += out[64:128] (SWDGE)
                nc.gpsimd.dma_start(out_sb[0:BATCH, :],
                                    out_sb[BATCH:2 * BATCH, :],
                                    accum_op=mybir.AluOpType.add)
                nc.sync.dma_start(out_d.ap(), out_sb[0:BATCH, :])

            if repeat == 1:
                body()
            else:
                with tc.For_i(0, repeat, 1):
                    body()

    nc.compile()
    return nc


def _build_nc_f32r(g=4, wbufs=3):
    """float32r W + exact hi/lo split of x, one PSUM chain (fallback)."""
    n_groups = N_KCHUNKS // g
    nc = bacc.Bacc("TRN2", target_bir_lowering=False, debug=False,
                   num_devices=N_CORES)
    xh_d, xl_d, wt_view, bias_d, out_d = _common_io(nc, F32R, g, F32)

    with tile.TileContext(nc) as tc:
        with (
            tc.tile_pool(name="const", bufs=1) as const_pool,
            tc.tile_pool(name="wpool", bufs=wbufs) as wpool,
            tc.tile_pool(name="psum", bufs=1, space=bass.MemorySpace.PSUM) as psum_pool,
            tc.tile_pool(name="opool", bufs=1) as opool,
        ):
            xh_sb = const_pool.tile([K_CHUNK, N_KCHUNKS * BATCH], F32R)
            nc.sync.dma_start(xh_sb[:], xh_d.ap())
            xl_sb = const_pool.tile([K_CHUNK, N_KCHUNKS * BATCH], F32R)
            nc.sync.dma_start(xl_sb[:], xl_d.ap())
            bias_sb = const_pool.tile([2, O_SHARD], F32)
            nc.sync.dma_start(bias_sb[:], bias_d.ap())
            ones_sb = const_pool.tile([1, BATCH], F32)
            nc.vector.memset(ones_sb[:], 1.0)

            psum = psum_pool.tile([BATCH, O_SHARD], F32)
            for oc in range(N_OCHUNKS):
                nc.tensor.matmul(
                    psum[:, oc * MM_FREE:(oc + 1) * MM_FREE],
                    ones_sb[:, :],
                    bias_sb[0:1, oc * MM_FREE:(oc + 1) * MM_FREE],
                    start=True, stop=False,
                )

            for g_idx in range(n_groups):
                w_sb = wpool.tile([K_CHUNK, g, O_SHARD], F32R)
                nc.sync.dma_start(w_sb[:], wt_view[g_idx])
                for j in range(g):
                    c = g_idx * g + j
                    lhs_hi = xh_sb[:, c * BATCH:(c + 1) * BATCH]
                    lhs_lo = xl_sb[:, c * BATCH:(c + 1) * BATCH]
                    last = c == N_KCHUNKS - 1
                    for oc in range(N_OCHUNKS):
                        rhs = w_sb[:, j, oc * MM_FREE:(oc + 1) * MM_FREE]
                        sl = slice(oc * MM_FREE, (oc + 1) * MM_FREE)
                        nc.tensor.matmul(psum[:, sl], lhs_hi, rhs,
                                         start=False, stop=False)
                        nc.tensor.matmul(psum[:, sl], lhs_lo, rhs,
                                         start=False, stop=last)

            out_sb = opool.tile([BATCH, O_SHARD], F32)
            nc.vector.tensor_copy(out_sb[:], psum[:])
            nc.sync.dma_start(out_d.ap(), out_sb[:])

    nc.compile()
    return nc


# winner of the HW sweeps: fp16 path, 2 batch-hunks, scalar-queue warm-up
_FAST_KW = {"nh": 2, "store_mode": "warm"}


def _get_nc(kind):
    if kind not in _compiled:
        if kind == "fast":
            _compiled[kind] = _build_nc_v4(**_FAST_KW)
        elif kind == "fp16":
            _compiled[kind] = _build_nc_fp16()
        else:
            _compiled[kind] = _build_nc_f32r()
    return _compiled[kind]


def _round_mantissa(a: np.ndarray, keep: int) -> np.ndarray:
    """Round fp32 mantissa to `keep` bits (round-to-nearest-even-ish at the
    boundary; carries into the exponent round correctly)."""
    u = a.view(np.uint32).astype(np.uint64)
    drop = 23 - keep
    rnd = ((u >> drop) & 1) + ((np.uint64(1) << np.uint64(drop - 1)) - np.uint64(1))
    u = ((u + rnd) >> np.uint64(drop)) << np.uint64(drop)
    return u.astype(np.uint32).view(np.float32)


def _xt_layout(x: np.ndarray) -> np.ndarray:
    """[B, NM] -> [128, N_KCHUNKS*BATCH] with [p, c*B + b] = x[b, c*128+p]."""
    return np.ascontiguousarray(
        x.reshape(BATCH, NM).T.reshape(N_KCHUNKS, K_CHUNK, BATCH)
        .transpose(1, 0, 2)
    ).reshape(K_CHUNK, N_KCHUNKS * BATCH)


def kernel(x, weight, bias):
    x = np.ascontiguousarray(x, dtype=np.float32)
    weight = np.ascontiguousarray(weight, dtype=np.float32)
    bias = np.ascontiguousarray(bias, dtype=np.float32)

    if _is_controlnet_weight(weight):
        in_maps = _v4_in_maps(x, bias)
        nc = _get_nc("fast")
        results = run_bass_kernel_spmd(nc, in_maps,
                                       core_ids=list(range(N_CORES))).results
        return _v4_unshard(results)

    xt_arr = _xt_layout(x)
    wt = weight.T  # [k, o] view
    wt_shards = [np.ascontiguousarray(wt[:, c * O_SHARD:(c + 1) * O_SHARD])
                 for c in range(N_CORES)]

    # fp16 fast path iff the weight is exactly fp16-representable
    # (true for this module's 0/1 permutation weight); exact f32r
    # split-x fallback otherwise.
    wt_f16 = [s.astype(np.float16) for s in wt_shards]
    exact = all(np.array_equal(h.astype(np.float32), s)
                for h, s in zip(wt_f16, wt_shards))

    if exact:
        x_hi32 = x.astype(np.float16).astype(np.float32)
        x_hi = _xt_layout(x_hi32).astype(np.float16)
        x_lo = _xt_layout((x - x_hi32) * float(2 ** LO_SHIFT)).astype(np.float16)
        b_hi32 = bias.astype(np.float16).astype(np.float32)
        b_lo = ((bias - b_hi32) * float(2 ** LO_SHIFT)).astype(np.float16)
        b2 = np.stack([b_hi32.astype(np.float16), b_lo])  # [2, NM] fp16
        in_maps = [{"xh": x_hi, "xl": x_lo, "wt": wt_f16[c],
                    "bias": np.ascontiguousarray(
                        b2[:, c * O_SHARD:(c + 1) * O_SHARD])}
                   for c in range(N_CORES)]
        nc = _get_nc("fp16")
    else:
        x_hi = _round_mantissa(xt_arr, 11)
        x_lo = xt_arr - x_hi  # exact in fp32
        b2 = np.stack([bias, np.zeros_like(bias)])  # [2, NM] f32; row 0 used
        in_maps = [{"xh": x_hi, "xl": x_lo, "wt": wt_shards[c],
                    "bias": np.ascontiguousarray(
                        b2[:, c * O_SHARD:(c + 1) * O_SHARD])}
                   for c in range(N_CORES)]
        nc = _get_nc("f32r")

    results = run_bass_kernel_spmd(nc, in_maps,
                                   core_ids=list(range(N_CORES))).results
    out = np.concatenate([r["out"] for r in results], axis=1)  # [64, 16384]
    return out.reshape(BATCH, 128, 128)



# revision 27
# speedup vs baseline: 1.0110x; 1.0110x over previous
"""Trainium2 Bass kernel for nn_ControlNet: out = x @ W^T + bias.

Shapes: x [64, 128, 128] f32, weight [16384, 16384] f32, bias [16384] f32.

Fast path (used when the weight matches the ControlNet structure,
verified on the host): the weight is a fixed block-diagonal 0/1 scatter
with 128 identical blocks -- each output row has at most one source:

  out[b, n, c] = x[b, n, 183 - c] + bias[n, c]   for c in [93, 120)
  out[b, n, c] = bias[n, c]                      otherwise

so the op is pure data movement plus one 27-column add; no matmul.
Sharding: data-parallel over batch (8 batches/core). Each core receives
only the 27-col band of x it needs plus bias (fp16, one 86KB DMA),
builds out[n, b, m] = broadcast(bias) on DVE with the band added on top
(fp16, stride-0 broadcast APs), and stores the 256KB fp16 result in two
batch-hunks on separate DMA queues (a tiny dummy store pre-warms the
second queue; a cold queue costs ~1us to first data). The host casts
back to f32 and unshards. Measured ~14.9us vs 268.5us for the dense
fp16 matmul baseline (fixed NEFF scaffold is ~10.5us of that); max rel
err ~5e-4 from fp16 rounding (gate: 2e-2).

Fallback (any other weight): dense tensor-parallel row-shard of W^T
across 8 cores streaming through the PE array -- fp16 with an exact
two-term hi/lo split of x when the weight is fp16-representable, else
float32r with the same split. See _build_nc_fp16/_build_nc_f32r.
"""

import numpy as np

import concourse.bacc as bacc
import concourse.bass as bass
import concourse.mybir as mybir
import concourse.tile as tile
from concourse.bass_utils import run_bass_kernel_spmd

BATCH = 64
NM = 128 * 128          # 16384 flattened features
N_CORES = 8
O_SHARD = NM // N_CORES  # 2048 output features per core
K_CHUNK = 128            # contraction handled 128 rows (partitions) at a time
N_KCHUNKS = NM // K_CHUNK  # 128
MM_FREE = 512            # psum bank limit: 512 fp32 outputs per matmul
N_OCHUNKS = O_SHARD // MM_FREE  # 4
LO_SHIFT = 11            # x_lo scale: 2^11 (fp16 mantissa width)

F32 = mybir.dt.float32
F32R = mybir.dt.float32r
F16 = mybir.dt.float16

_compiled = {}

# --- fast path: the ControlNet weight is a fixed block-diagonal 0/1 scatter:
#   out[b, n, c] = x[b, n, 183 - c] + bias[n, c]   for c in [93, 120)
#   out[b, n, c] = bias[n, c]                      otherwise
# (128 identical blocks; 27 ones per block; 3456 nonzeros total).
# No matmul needed: per core (batch-shard of 8) the device reads the 27-col
# band of x + bias (one 86 KB fp16 DMA), builds out = broadcast(bias) with
# the band added on DVE, and stores 256 KB fp16. The single-shot NEFF time
# is dominated by the ~10.5us fixed scaffold (start barrier, per-engine
# library loads, exit barriers); the body is ~4.5us.
BAND_LO = 93
BAND_HI = 120
BAND_W = BAND_HI - BAND_LO  # 27
B_CORE = BATCH // N_CORES   # 8 batches per core


def _expected_pattern():
    n = np.repeat(np.arange(128), BAND_W)
    j = np.tile(np.arange(37, 64), 128)
    return 128 * n + j + 56, 128 * n + 127 - j


def _is_controlnet_weight(weight):
    if weight.shape != (NM, NM):
        return False
    if np.count_nonzero(weight) != 3456:
        return False
    oi, ii = _expected_pattern()
    return bool(np.all(weight[oi, ii] == 1.0))


def _build_nc_v4(nh=2, out_dt=F16, in_dt=F16, split_input=False,
                 store_mode="alt", num_devices=N_CORES, hb_list=None,
                 no_waw=False):
    """All-fp16 data path: fp16 inputs (band 216 cols + bias 128 cols in
    one DRAM tensor, one DMA on the sync ring), fp16 DVE compute (2x
    rate), fp16 output (half the store bytes). nh batch-hunks, stores
    alternating sync/scalar rings. store_mode "warm" issues a tiny dummy
    store on scalar first so its DMA queue is warm before the real store
    (a cold queue costs ~1us to first data; measured best + most stable:
    nh=2 + warm). split_input / sync2 were slower in HW sweeps.
    """
    hb = B_CORE // nh
    nc = bacc.Bacc("TRN2", target_bir_lowering=False, debug=False,
                   num_devices=num_devices)
    BCOLS = B_CORE * BAND_W  # 216
    in_d = nc.dram_tensor("inp", [128, BCOLS + 128], in_dt,
                          kind="ExternalInput")
    out_d = nc.dram_tensor("out", [128, B_CORE, 128], out_dt,
                           kind="ExternalOutput")
    if store_mode == "warm":
        scratch_d = nc.dram_tensor("scratch", [1, 16], in_dt,
                                   kind="Internal")

    with tile.TileContext(nc) as tc:
        with tc.tile_pool(name="pool", bufs=1) as pool:
            in_sb = pool.tile([128, BCOLS + 128], in_dt)
            if store_mode == "warm":
                # tiny write to warm the scalar DMA queue before the
                # real store needs it (cold queue: ~1us to first data)
                warm_sb = pool.tile([1, 16], in_dt)
                nc.vector.memset(warm_sb[:], 0.0)
                nc.scalar.dma_start(scratch_d.ap(), warm_sb[:])
            if split_input:
                nc.sync.dma_start(in_sb[:, BCOLS:BCOLS + 128],
                                  in_d.ap()[:, BCOLS:BCOLS + 128])
                nc.scalar.dma_start(in_sb[:, 0:BCOLS],
                                    in_d.ap()[:, 0:BCOLS])
            else:
                nc.sync.dma_start(in_sb[:], in_d.ap())
            band = in_sb[:, 0:BCOLS].rearrange("p (b k) -> p b k", b=B_CORE)
            bias = in_sb[:, BCOLS:BCOLS + 128]
            if store_mode == "sync2":
                rings = [nc.sync, nc.sync]
            else:
                rings = [nc.sync, nc.scalar]

            out_sb = pool.tile([128, B_CORE, 128], out_dt)
            bounds = [0]
            for w in (hb_list or [hb] * nh):
                bounds.append(bounds[-1] + w)
            assert bounds[-1] == B_CORE
            for h in range(len(bounds) - 1):
                bsl = slice(bounds[h], bounds[h + 1])
                w = bounds[h + 1] - bounds[h]
                if no_waw:
                    # copy only the non-band columns so the band add has
                    # no WAW dependency on the copy -- the scheduler can
                    # run them in any order and the store issues sooner
                    nc.vector.tensor_copy(
                        out_sb[:, bsl, 0:BAND_LO],
                        bias[:, 0:BAND_LO].unsqueeze(1)
                        .broadcast_to([128, w, BAND_LO]))
                    nc.vector.tensor_copy(
                        out_sb[:, bsl, BAND_HI:128],
                        bias[:, BAND_HI:128].unsqueeze(1)
                        .broadcast_to([128, w, 128 - BAND_HI]))
                else:
                    nc.vector.tensor_copy(
                        out_sb[:, bsl, :],
                        bias.unsqueeze(1).broadcast_to([128, w, 128]))
                nc.vector.tensor_add(
                    out_sb[:, bsl, BAND_LO:BAND_HI],
                    band[:, bsl, :],
                    bias[:, BAND_LO:BAND_HI].unsqueeze(1)
                    .broadcast_to([128, w, BAND_W]))
                rings[h % 2].dma_start(out_d.ap()[:, bsl, :],
                                       out_sb[:, bsl, :])

    nc.compile()
    return nc


def _v4_in_maps(x, bias, in_np=np.float16):
    band = x[:, :, 90:63:-1]  # [64, 128, 27]
    bias2d = bias.reshape(128, 128)
    maps = []
    for c in range(N_CORES):
        bc = band[c * B_CORE:(c + 1) * B_CORE].transpose(1, 0, 2)  # [128,8,27]
        inp = np.empty((128, B_CORE * BAND_W + 128), dtype=in_np)
        inp[:, 0:B_CORE * BAND_W] = bc.reshape(128, -1)
        inp[:, B_CORE * BAND_W:] = bias2d
        maps.append({"inp": inp})
    return maps


def _v4_unshard(results):
    out = np.concatenate(
        [r["out"].astype(np.float32).transpose(1, 0, 2) for r in results],
        axis=0)
    return np.ascontiguousarray(out)


def _common_io(nc, mm_dt, g, bias_dt):
    n_groups = N_KCHUNKS // g
    xh_d = nc.dram_tensor("xh", [K_CHUNK, N_KCHUNKS * BATCH], mm_dt,
                          kind="ExternalInput")
    xl_d = nc.dram_tensor("xl", [K_CHUNK, N_KCHUNKS * BATCH], mm_dt,
                          kind="ExternalInput")
    wt_d = nc.dram_tensor("wt", [NM, O_SHARD], mm_dt, kind="ExternalInput")
    bias_d = nc.dram_tensor("bias", [2, O_SHARD], bias_dt,
                            kind="ExternalInput")
    out_d = nc.dram_tensor("out", [BATCH, O_SHARD], F32, kind="ExternalOutput")
    # W^T shard grouped for DMA: k = (g_idx*g + j)*128 + p  ->  [g_idx, p, j, o]
    wt_view = wt_d.ap().rearrange("(g j p) o -> g p j o", g=n_groups, j=g,
                                  p=K_CHUNK)
    return xh_d, xl_d, wt_view, bias_d, out_d


def _build_nc_fp16(g=8, wbufs=3, repeat=1):
    """fp16 W + exact fp16 hi/lo split of x, two PSUM chains.

    Every PE instruction is fp16 (the fp32/fp16 mix crashed the exec
    unit): bias is split like x, bias_hi into the hi chain and
    bias_lo * 2^11 into the lo chain, each as the chain-starting
    contract-dim-1 matmul.

    repeat > 1 wraps the streaming body in a device-side For_i loop —
    used only for benchmarking (per-call dispatch overhead through the
    axon tunnel is ~88 ms, so single executions can't be timed).
    """
    n_groups = N_KCHUNKS // g
    nc = bacc.Bacc("TRN2", target_bir_lowering=False, debug=False,
                   num_devices=N_CORES)
    xh_d, xl_d, wt_view, bias_d, out_d = _common_io(nc, F16, g, F16)

    with tile.TileContext(nc) as tc:
        with (
            tc.tile_pool(name="const", bufs=1) as const_pool,
            tc.tile_pool(name="wpool", bufs=wbufs) as wpool,
            tc.tile_pool(name="psum", bufs=1, space=bass.MemorySpace.PSUM) as psum_pool,
            tc.tile_pool(name="opool", bufs=1) as opool,
        ):
            xh_sb = const_pool.tile([K_CHUNK, N_KCHUNKS * BATCH], F16)
            nc.sync.dma_start(xh_sb[:], xh_d.ap())
            xl_sb = const_pool.tile([K_CHUNK, N_KCHUNKS * BATCH], F16)
            nc.sync.dma_start(xl_sb[:], xl_d.ap())
            bias_hi_sb = const_pool.tile([1, O_SHARD], F16)
            nc.sync.dma_start(bias_hi_sb[:], bias_d.ap()[0:1])
            bias_lo_sb = const_pool.tile([1, O_SHARD], F16)
            nc.sync.dma_start(bias_lo_sb[:], bias_d.ap()[1:2])
            ones_sb = const_pool.tile([1, BATCH], F16)
            nc.vector.memset(ones_sb[:], 1.0)

            def body():
                psum_hi = psum_pool.tile([BATCH, O_SHARD], F32, tag="ph")
                psum_lo = psum_pool.tile([BATCH, O_SHARD], F32, tag="pl")
                # bias rows into each chain: [1,64].T @ [1,512] outer product
                for oc in range(N_OCHUNKS):
                    sl = slice(oc * MM_FREE, (oc + 1) * MM_FREE)
                    nc.tensor.matmul(psum_hi[:, sl], ones_sb[:, :],
                                     bias_hi_sb[0:1, sl], start=True, stop=False)
                    nc.tensor.matmul(psum_lo[:, sl], ones_sb[:, :],
                                     bias_lo_sb[0:1, sl], start=True, stop=False)

                for g_idx in range(n_groups):
                    w_sb = wpool.tile([K_CHUNK, g, O_SHARD], F16, tag="w")
                    nc.sync.dma_start(w_sb[:], wt_view[g_idx])
                    for j in range(g):
                        c = g_idx * g + j
                        lhs_hi = xh_sb[:, c * BATCH:(c + 1) * BATCH]
                        lhs_lo = xl_sb[:, c * BATCH:(c + 1) * BATCH]
                        last = c == N_KCHUNKS - 1
                        for oc in range(N_OCHUNKS):
                            rhs = w_sb[:, j, oc * MM_FREE:(oc + 1) * MM_FREE]
                            sl = slice(oc * MM_FREE, (oc + 1) * MM_FREE)
                            nc.tensor.matmul(psum_hi[:, sl], lhs_hi, rhs,
                                             start=False, stop=last)
                            nc.tensor.matmul(psum_lo[:, sl], lhs_lo, rhs,
                                             start=False, stop=last)

                out_sb = opool.tile([BATCH, O_SHARD], F32, tag="o")
                # out = (lo * 2^-11) + hi (DVE reads <=1 PSUM input per op)
                nc.vector.tensor_scalar_mul(out_sb[:], psum_lo[:],
                                            2.0 ** -LO_SHIFT)
                nc.vector.tensor_add(out_sb[:], out_sb[:], psum_hi[:])
                nc.sync.dma_start(out_d.ap(), out_sb[:])

            if repeat == 1:
                body()
            else:
                with tc.For_i(0, repeat, 1):
                    body()

    nc.compile()
    return nc


def _build_nc_fp16ct(g=8, wbufs=3, repeat=1, const_engine=None, dual_ring=False):
    """Column-tiled fp16 variant: hi chain on PE columns 0-63
    (tile_position (0,0), PSUM partitions 0-63), lo chain on columns
    64-127 (tile_position (0,64), PSUM partitions 64-127). The two
    matmuls of each k-chunk run concurrently on disjoint column groups,
    halving effective PE time. The tail merges across partitions with an
    SBUF->SBUF accumulate DMA (SWDGE)."""
    n_groups = N_KCHUNKS // g
    nc = bacc.Bacc("TRN2", target_bir_lowering=False, debug=False,
                   num_devices=N_CORES)
    xh_d, xl_d, wt_view, bias_d, out_d = _common_io(nc, F16, g, F16)

    with tile.TileContext(nc) as tc:
        with (
            tc.tile_pool(name="const", bufs=1) as const_pool,
            tc.tile_pool(name="wpool", bufs=wbufs) as wpool,
            tc.tile_pool(name="psum", bufs=1, space=bass.MemorySpace.PSUM) as psum_pool,
            tc.tile_pool(name="opool", bufs=1) as opool,
        ):
            ce = nc.scalar if const_engine == "scalar" else nc.sync
            xh_sb = const_pool.tile([K_CHUNK, N_KCHUNKS * BATCH], F16)
            ce.dma_start(xh_sb[:], xh_d.ap())
            xl_sb = const_pool.tile([K_CHUNK, N_KCHUNKS * BATCH], F16)
            ce.dma_start(xl_sb[:], xl_d.ap())
            bias_hi_sb = const_pool.tile([1, O_SHARD], F16)
            ce.dma_start(bias_hi_sb[:], bias_d.ap()[0:1])
            bias_lo_sb = const_pool.tile([1, O_SHARD], F16)
            ce.dma_start(bias_lo_sb[:], bias_d.ap()[1:2])
            ones_sb = const_pool.tile([1, BATCH], F16)
            nc.vector.memset(ones_sb[:], 1.0)

            def body():
                # separate banks per chain: hi banks 0-3 (partitions 0-63),
                # lo banks 4-7 (partitions 64-127, via col-group 2-3)
                psum_hi = psum_pool.tile([BATCH, O_SHARD], F32, tag="ph")
                psum_lo = psum_pool.tile([2 * BATCH, O_SHARD], F32, tag="pl")
                for oc in range(N_OCHUNKS):
                    sl = slice(oc * MM_FREE, (oc + 1) * MM_FREE)
                    nc.tensor.matmul(psum_hi[:, sl], ones_sb[:, :],
                                     bias_hi_sb[0:1, sl], start=True,
                                     stop=False, tile_position=(0, 0))
                    nc.tensor.matmul(psum_lo[BATCH:2 * BATCH, sl],
                                     ones_sb[:, :],
                                     bias_lo_sb[0:1, sl], start=True,
                                     stop=False, tile_position=(0, 64))

                for g_idx in range(n_groups):
                    w_sb = wpool.tile([K_CHUNK, g, O_SHARD], F16, tag="w")
                    weng = (nc.scalar if (dual_ring and g_idx % 2) else nc.sync)
                    weng.dma_start(w_sb[:], wt_view[g_idx])
                    for j in range(g):
                        c = g_idx * g + j
                        lhs_hi = xh_sb[:, c * BATCH:(c + 1) * BATCH]
                        lhs_lo = xl_sb[:, c * BATCH:(c + 1) * BATCH]
                        last = c == N_KCHUNKS - 1
                        for oc in range(N_OCHUNKS):
                            rhs = w_sb[:, j, oc * MM_FREE:(oc + 1) * MM_FREE]
                            sl = slice(oc * MM_FREE, (oc + 1) * MM_FREE)
                            nc.tensor.matmul(psum_hi[:, sl], lhs_hi, rhs,
                                             start=False, stop=last,
                                             tile_position=(0, 0))
                            nc.tensor.matmul(psum_lo[BATCH:2 * BATCH, sl],
                                             lhs_lo, rhs,
                                             start=False, stop=last,
                                             tile_position=(0, 64))

                out_sb = opool.tile([2 * BATCH, O_SHARD], F32, tag="o")
                # rows 64-127: lo * 2^-11 ; rows 0-63: hi
                nc.vector.tensor_scalar_mul(out_sb[BATCH:2 * BATCH, :],
                                            psum_lo[BATCH:2 * BATCH, :],
                                            2.0 ** -LO_SHIFT)
                nc.vector.tensor_copy(out_sb[0:BATCH, :], psum_hi[:, :])
                # cross-partition merge: out[0:64] += out[64:128] (SWDGE)
                nc.gpsimd.dma_start(out_sb[0:BATCH, :],
                                    out_sb[BATCH:2 * BATCH, :],
                                    accum_op=mybir.AluOpType.add)
                nc.sync.dma_start(out_d.ap(), out_sb[0:BATCH, :])

            if repeat == 1:
                body()
            else:
                with tc.For_i(0, repeat, 1):
                    body()

    nc.compile()
    return nc


def _build_nc_f32r(g=4, wbufs=3):
    """float32r W + exact hi/lo split of x, one PSUM chain (fallback)."""
    n_groups = N_KCHUNKS // g
    nc = bacc.Bacc("TRN2", target_bir_lowering=False, debug=False,
                   num_devices=N_CORES)
    xh_d, xl_d, wt_view, bias_d, out_d = _common_io(nc, F32R, g, F32)

    with tile.TileContext(nc) as tc:
        with (
            tc.tile_pool(name="const", bufs=1) as const_pool,
            tc.tile_pool(name="wpool", bufs=wbufs) as wpool,
            tc.tile_pool(name="psum", bufs=1, space=bass.MemorySpace.PSUM) as psum_pool,
            tc.tile_pool(name="opool", bufs=1) as opool,
        ):
            xh_sb = const_pool.tile([K_CHUNK, N_KCHUNKS * BATCH], F32R)
            nc.sync.dma_start(xh_sb[:], xh_d.ap())
            xl_sb = const_pool.tile([K_CHUNK, N_KCHUNKS * BATCH], F32R)
            nc.sync.dma_start(xl_sb[:], xl_d.ap())
            bias_sb = const_pool.tile([2, O_SHARD], F32)
            nc.sync.dma_start(bias_sb[:], bias_d.ap())
            ones_sb = const_pool.tile([1, BATCH], F32)
            nc.vector.memset(ones_sb[:], 1.0)

            psum = psum_pool.tile([BATCH, O_SHARD], F32)
            for oc in range(N_OCHUNKS):
                nc.tensor.matmul(
                    psum[:, oc * MM_FREE:(oc + 1) * MM_FREE],
                    ones_sb[:, :],
                    bias_sb[0:1, oc * MM_FREE:(oc + 1) * MM_FREE],
                    start=True, stop=False,
                )

            for g_idx in range(n_groups):
                w_sb = wpool.tile([K_CHUNK, g, O_SHARD], F32R)
                nc.sync.dma_start(w_sb[:], wt_view[g_idx])
                for j in range(g):
                    c = g_idx * g + j
                    lhs_hi = xh_sb[:, c * BATCH:(c + 1) * BATCH]
                    lhs_lo = xl_sb[:, c * BATCH:(c + 1) * BATCH]
                    last = c == N_KCHUNKS - 1
                    for oc in range(N_OCHUNKS):
                        rhs = w_sb[:, j, oc * MM_FREE:(oc + 1) * MM_FREE]
                        sl = slice(oc * MM_FREE, (oc + 1) * MM_FREE)
                        nc.tensor.matmul(psum[:, sl], lhs_hi, rhs,
                                         start=False, stop=False)
                        nc.tensor.matmul(psum[:, sl], lhs_lo, rhs,
                                         start=False, stop=last)

            out_sb = opool.tile([BATCH, O_SHARD], F32)
            nc.vector.tensor_copy(out_sb[:], psum[:])
            nc.sync.dma_start(out_d.ap(), out_sb[:])

    nc.compile()
    return nc


# winner of the HW sweeps: fp16 path, 2 batch-hunks, scalar-queue
# warm-up, band-add independent of the bias copies (no WAW)
_FAST_KW = {"nh": 2, "store_mode": "warm", "no_waw": True}


def _get_nc(kind):
    if kind not in _compiled:
        if kind == "fast":
            _compiled[kind] = _build_nc_v4(**_FAST_KW)
        elif kind == "fp16":
            _compiled[kind] = _build_nc_fp16()
        else:
            _compiled[kind] = _build_nc_f32r()
    return _compiled[kind]


def _round_mantissa(a: np.ndarray, keep: int) -> np.ndarray:
    """Round fp32 mantissa to `keep` bits (round-to-nearest-even-ish at the
    boundary; carries into the exponent round correctly)."""
    u = a.view(np.uint32).astype(np.uint64)
    drop = 23 - keep
    rnd = ((u >> drop) & 1) + ((np.uint64(1) << np.uint64(drop - 1)) - np.uint64(1))
    u = ((u + rnd) >> np.uint64(drop)) << np.uint64(drop)
    return u.astype(np.uint32).view(np.float32)


def _xt_layout(x: np.ndarray) -> np.ndarray:
    """[B, NM] -> [128, N_KCHUNKS*BATCH] with [p, c*B + b] = x[b, c*128+p]."""
    return np.ascontiguousarray(
        x.reshape(BATCH, NM).T.reshape(N_KCHUNKS, K_CHUNK, BATCH)
        .transpose(1, 0, 2)
    ).reshape(K_CHUNK, N_KCHUNKS * BATCH)


def kernel(x, weight, bias):
    x = np.ascontiguousarray(x, dtype=np.float32)
    weight = np.ascontiguousarray(weight, dtype=np.float32)
    bias = np.ascontiguousarray(bias, dtype=np.float32)

    if _is_controlnet_weight(weight):
        in_maps = _v4_in_maps(x, bias)
        nc = _get_nc("fast")
        results = run_bass_kernel_spmd(nc, in_maps,
                                       core_ids=list(range(N_CORES))).results
        return _v4_unshard(results)

    xt_arr = _xt_layout(x)
    wt = weight.T  # [k, o] view
    wt_shards = [np.ascontiguousarray(wt[:, c * O_SHARD:(c + 1) * O_SHARD])
                 for c in range(N_CORES)]

    # fp16 fast path iff the weight is exactly fp16-representable
    # (true for this module's 0/1 permutation weight); exact f32r
    # split-x fallback otherwise.
    wt_f16 = [s.astype(np.float16) for s in wt_shards]
    exact = all(np.array_equal(h.astype(np.float32), s)
                for h, s in zip(wt_f16, wt_shards))

    if exact:
        x_hi32 = x.astype(np.float16).astype(np.float32)
        x_hi = _xt_layout(x_hi32).astype(np.float16)
        x_lo = _xt_layout((x - x_hi32) * float(2 ** LO_SHIFT)).astype(np.float16)
        b_hi32 = bias.astype(np.float16).astype(np.float32)
        b_lo = ((bias - b_hi32) * float(2 ** LO_SHIFT)).astype(np.float16)
        b2 = np.stack([b_hi32.astype(np.float16), b_lo])  # [2, NM] fp16
        in_maps = [{"xh": x_hi, "xl": x_lo, "wt": wt_f16[c],
                    "bias": np.ascontiguousarray(
                        b2[:, c * O_SHARD:(c + 1) * O_SHARD])}
                   for c in range(N_CORES)]
        nc = _get_nc("fp16")
    else:
        x_hi = _round_mantissa(xt_arr, 11)
        x_lo = xt_arr - x_hi  # exact in fp32
        b2 = np.stack([bias, np.zeros_like(bias)])  # [2, NM] f32; row 0 used
        in_maps = [{"xh": x_hi, "xl": x_lo, "wt": wt_shards[c],
                    "bias": np.ascontiguousarray(
                        b2[:, c * O_SHARD:(c + 1) * O_SHARD])}
                   for c in range(N_CORES)]
        nc = _get_nc("f32r")

    results = run_bass_kernel_spmd(nc, in_maps,
                                   core_ids=list(range(N_CORES))).results
    out = np.concatenate([r["out"] for r in results], axis=1)  # [64, 16384]
    return out.reshape(BATCH, 128, 128)

